# revision 6
# baseline (speedup 1.0000x reference)
"""Block-sparse linear y = x @ W^T on 8 Trainium2 NeuronCores.

Strategy: the 32x32 block structure (50% block density, random scatter) is not
exploitable on a 128x128 PE array (any packing at 32-granularity wastes more
PE volume than the ~39% merged density saves), so we densify W^T on the host
(cheap: 8MB of scatter-adds) and run a dense GEMM, sharded 4-way over tokens
x 2-way over out_features (8 cores, no collectives).

The matmuls run in bfloat16 (PE rate is identical to float32r at 1.0
cycles/row, so the PE floor is 131072 cycles = 54.6us/core either way), which
halves input HBM traffic to 8MB/core (~24us at effective DMA bandwidth) and
takes the DMA stream well off the PE's critical path — the fp32r version sat
exactly at the 20MB/55us ridge and any DMA jitter stalled the PE. bf16
rounding error is ~1.3e-3 max-rel vs the fp32 reference (tolerance 2e-2).

Schedule per core, two passes over the out-feature halves:
(1) n=0, k-outer/m-inner: x k-tiles and W k-pair tiles stream just-in-time
    (384KB per 1.7us k-step, ~2/3 of DMA supply); all 8 psum banks
    accumulate the full K.
(2) n=1, m-outer/k-inner: everything is SBUF-resident by now (the n=1 W half
    prefetches as two 1MB DMAs behind the n=0 stream), so each bank runs its
    16 matmuls back-to-back and drains (vector copy + output DMA) while the
    next bank computes; bank evictions never collide with the PE.
Outputs ride the scalar-engine queue so y stores can't delay input loads.
A few dummy matmuls at the start warm the PE clock gate during the initial
DMA head wait.
"""

import numpy as np

TOKENS, IN_F, OUT_F = 4096, 2048, 2048
BLOCK = 32
N_CORES = 8
TG, OG = 4, 2  # token groups x out-feature groups
T_SH = TOKENS // TG  # 1024 tokens per core
O_SH = OUT_F // OG  # 1024 out features per core
P = 128
NFREE = 512  # PSUM bank free dim (fp32)
KT = IN_F // P  # 16 k tiles
MT = T_SH // P  # 8 psum banks
XH = T_SH // 2  # token half (k=0 ramp split)

MM_DTYPE = "bfloat16"  # "bfloat16" (fast DMA) or "float32r" (exact-ish)
TRACE = False  # set by test.py to capture an NTFF profile

_nc_cache = {}
_last_result = None  # BassKernelResults of the most recent run (for test.py)


def _build_nc():
    import concourse.mybir as mybir
    import concourse.tile as tile
    from concourse import bacc

    key = MM_DTYPE
    if key in _nc_cache:
        return _nc_cache[key]

    dt_mm = getattr(mybir.dt, MM_DTYPE)
    f32 = mybir.dt.float32

    nc = bacc.Bacc(None, target_bir_lowering=False)
    # Host-pre-blocked inputs (exact SBUF layouts; all DMAs are linear):
    # xt: x^T k-tiles, [KT][P][T_SH]
    # wq: W^T halves by out-column, [2][P][KT][NFREE]
    xt = nc.dram_tensor("xt", [KT, P, T_SH], dt_mm, kind="ExternalInput")
    wq = nc.dram_tensor("wq", [2, P, KT, NFREE], dt_mm, kind="ExternalInput")
    y = nc.dram_tensor("y", [T_SH, O_SH], f32, kind="ExternalOutput")

    with tile.TileContext(nc) as tc:
        with (
            tc.tile_pool(name="xp", bufs=1) as xp,
            tc.tile_pool(name="wp", bufs=1) as wp,
            tc.tile_pool(name="op", bufs=1) as op,
            tc.tile_pool(name="ps", bufs=1, space="PSUM") as ps,
        ):
            # Warm the PE's HAM clock gate during the initial DMA head wait:
            # short dummy matmuls (tiny memset dependency so they can start
            # the moment the PE queue drains its preamble) keep the array
            # busy so the first real matmuls run at full clock.
            zt = xp.tile([P, P], dt_mm, tag="warm", name="warm")
            nc.gpsimd.memset(zt[:], 0.0)
            warm_ps = ps.tile([P, NFREE], f32, tag="ps0", name="warm_ps")
            for _ in range(13):
                nc.tensor.matmul(warm_ps[:, :P], zt[:], zt[:], start=True, stop=True)

            xa = [None] * KT  # [P, T_SH] x^T tiles (k=0: two token-halves)
            x0h = [None, None]

            def lhsT(m, k):
                """Stationary x^T slice for bank m, k-tile k."""
                if k == 0:
                    return x0h[m // 4][:, (m % 4) * P : (m % 4 + 1) * P]
                return xa[k][:, m * P : (m + 1) * P]

            def psums():
                return [
                    ps.tile([P, NFREE], f32, tag=f"ps{m}", name=f"ps{m}")
                    for m in range(MT)
                ]

            # Head: the k=0/k=1 W tiles ride the scalar-engine HWDGE queue so
            # their issue overlaps the sync queue's x loads — the two
            # sequencers each take ~0.6us per dma_start, which otherwise
            # serializes in front of the first matmul.
            w01 = []
            for k in range(2):
                wt = wp.tile([P, 1, NFREE], dt_mm, tag=f"w0_{k}", name=f"w0_{k}")
                nc.scalar.dma_start(wt[:], wq[0, :, k : k + 1, :])
                w01.append(wt)

            # ---- Pass 1: n=0, k-outer/m-inner, x + W streamed JIT ----
            ps0 = psums()
            w0 = []
            for k in range(KT):
                if k == 0:  # first x tile in token-halves for a fast ramp
                    for h in range(2):
                        t = xp.tile([P, XH], dt_mm, tag=f"x0_{h}", name=f"x0_{h}")
                        nc.sync.dma_start(t[:], xt[0, :, h * XH : (h + 1) * XH])
                        x0h[h] = t
                else:
                    if k >= 2 and k % 2 == 0:
                        # remaining W k-tiles in pairs: 2KB contiguous runs
                        wt = wp.tile(
                            [P, 2, NFREE], dt_mm, tag=f"w0_{k}", name=f"w0_{k}"
                        )
                        nc.sync.dma_start(wt[:], wq[0, :, k : k + 2, :])
                        w0.append(wt)
                    t = xp.tile([P, T_SH], dt_mm, tag=f"x{k}", name=f"x{k}")
                    nc.sync.dma_start(t[:], xt[k])
                    xa[k] = t
                for m in range(MT):
                    rhs = (
                        w01[k][:, 0, :]
                        if k < 2
                        else w0[k // 2 - 1][:, k % 2, :]
                    )
                    nc.tensor.matmul(
                        ps0[m][:],
                        lhsT(m, k),
                        rhs,
                        start=(k == 0),
                        stop=(k == KT - 1),
                    )

            # n=1 W half: two 1MB prefetches queued behind the n=0 stream
            w1 = []
            for h in range(2):
                wt = wp.tile([P, KT // 2, NFREE], dt_mm, tag=f"w1_{h}", name=f"w1_{h}")
                nc.sync.dma_start(
                    wt[:], wq[1, :, h * (KT // 2) : (h + 1) * (KT // 2), :]
                )
                w1.append(wt)

            for m in range(MT):  # evict n=0 psums; y stores on the scalar queue
                ot = op.tile([P, NFREE], f32, tag=f"o0_{m}", name=f"o0_{m}")
                nc.vector.tensor_copy(ot[:], ps0[m][:])
                nc.scalar.dma_start(y[m * P : (m + 1) * P, 0:NFREE], ot[:])

            # ---- Pass 2: n=1, m-outer/k-inner; each bank drains as it ends ----
            ps1 = psums()
            for m in range(MT):
                for k in range(KT):
                    nc.tensor.matmul(
                        ps1[m][:],
                        lhsT(m, k),
                        w1[k // (KT // 2)][:, k % (KT // 2), :],
                        start=(k == 0),
                        stop=(k == KT - 1),
                    )
                ot = op.tile([P, NFREE], f32, tag=f"o1_{m}", name=f"o1_{m}")
                if m == MT - 1:
                    # last bank: drain in halves so the copy, store issue, and
                    # wire time pipeline instead of stacking up on the tail
                    for h in range(2):
                        sl = slice(h * (NFREE // 2), (h + 1) * (NFREE // 2))
                        nc.vector.tensor_copy(ot[:, sl], ps1[m][:, sl])
                        nc.scalar.dma_start(
                            y[m * P : (m + 1) * P, NFREE + h * (NFREE // 2) :
                              NFREE + (h + 1) * (NFREE // 2)],
                            ot[:, sl],
                        )
                else:
                    nc.vector.tensor_copy(ot[:], ps1[m][:])
                    nc.scalar.dma_start(
                        y[m * P : (m + 1) * P, NFREE : 2 * NFREE], ot[:]
                    )

    nc.compile()
    _nc_cache[key] = nc
    return nc


def _densify_wT(weight_blocks, block_rows, block_cols):
    """Scatter-add the 32x32 blocks into dense W^T [in_features, out_features]."""
    nc_blk = IN_F // BLOCK
    nr_blk = OUT_F // BLOCK
    wcr = np.zeros((nc_blk, nr_blk, BLOCK, BLOCK), np.float32)
    # block b occupies W[32r:32r+32, 32c:32c+32]; W^T gets the transposed block
    np.add.at(
        wcr,
        (block_cols.astype(np.int64), block_rows.astype(np.int64)),
        np.swapaxes(weight_blocks.astype(np.float32, copy=False), 1, 2),
    )
    return np.ascontiguousarray(wcr.transpose(0, 2, 1, 3).reshape(IN_F, OUT_F))


def _mm_np_dtype():
    if MM_DTYPE == "bfloat16":
        import ml_dtypes

        return np.dtype(ml_dtypes.bfloat16)
    return np.dtype(np.float32)


def _pack_core_inputs(xT_sh, wT_sh):
    """Block one core's x^T and W^T shards into the kernel's DMA layouts."""
    dt = _mm_np_dtype()
    xt = np.ascontiguousarray(xT_sh.reshape(KT, P, T_SH)).astype(dt)
    wq = np.ascontiguousarray(
        wT_sh.reshape(KT, P, 2, NFREE).transpose(2, 1, 0, 3)
    ).astype(dt)
    return {"xt": xt, "wq": wq}


def kernel(x, weight_blocks, block_rows, block_cols):
    global _last_result
    from concourse.bass_utils import run_bass_kernel_spmd

    x = np.asarray(x, dtype=np.float32)
    wT = _densify_wT(
        np.asarray(weight_blocks), np.asarray(block_rows), np.asarray(block_cols)
    )
    xT = np.ascontiguousarray(x.T)

    in_maps = []
    for c in range(N_CORES):
        tg, og = divmod(c, OG)
        in_maps.append(
            _pack_core_inputs(
                xT[:, tg * T_SH : (tg + 1) * T_SH],
                wT[:, og * O_SH : (og + 1) * O_SH],
            )
        )

    nc = _build_nc()
    res = None
    for attempt in range(3):  # transient NRT device errors happen; retry
        try:
            res = run_bass_kernel_spmd(
                nc, in_maps, core_ids=list(range(N_CORES)), trace=TRACE
            )
            break
        except Exception:
            if attempt == 2:
                raise
            import time

            time.sleep(3)
    _last_result = res

    y = np.empty((TOKENS, OUT_F), np.float32)
    for c in range(N_CORES):
        tg, og = divmod(c, OG)
        y[tg * T_SH : (tg + 1) * T_SH, og * O_SH : (og + 1) * O_SH] = res.results[c][
            "y"
        ]
    return y
